# revision 53
# baseline (speedup 1.0000x reference)
"""AdvancedFeatureTransformer Trainium2 kernel (bf16 + fp8 stats, interleaved).

Data-parallel over batch: 8 cores x 512 rows, no collectives.
All activations feature-major (h^T: [feat_part, batch_free]); matmul
operands bf16 (1 cycle/row), psum accumulates fp32.

Key structure (v3):
  - Attention at seq_len=1 is affine: Wa = Wo@Wv, ba = Wo@bv + bo are
    collapsed on the host, removing the v-projection stage per layer.
  - The cross-attention block (also affine) is folded into the 424 head
    W1 matrices on the host: W1eff = W1@(I + Wco@Wcv).
  - EVERY matmul is a full (128,128) PE tile: small stationaries (trunk
    LN stats/bcast, head W2/W3/b2/selDR) are zero-padded, because the HW
    charges ~110ns to switch PE tile configs between matmuls of
    different (row,col) shapes (measured 216ns same-shape vs 330ns
    switching for 512-col bf16 matmuls).
  - Trunk runs two 256-column batch halves interleaved one layer apart,
    so the halves' sqrt/gelu table uses cluster (2 ACT table loads per
    layer, hoisted off the LN chain by data-anchored warm activations).
  - Trunk LN stats matmul vs a full-ones stationary makes the psum
    output the partition-broadcast sum of squares, so sqrt/recip give
    broadcast rstd directly (no bcast matmul, no bf16 round trip).
  - Layer-0 LN mean-correction (-c) rides the attn psum via an idnb
    matmul of the broadcast c tile (no gpsimd broadcast).
  - Head LN stats use fp8e4 squares and a DoubleRow selection matmul:
    one instruction streams TWO targets' squares at 2 cols/cycle.
  - Head W1 psum is a [128, 2, 512] pair tile (fewer semaphore waits);
    relu on DVE, square (fp8 out) on ACT read the psum directly; phase
    A of group g is interleaved with phase B of group g-1 at pair
    granularity to balance PE/ACT/DVE.
  - rstd deferral for heads: relu(Z*r + b2) = r*relu(Z + b2*sd); the
    rank-1 b2 (x) sd term is a (zero-padded) K=128 matmul against the
    bank-broadcast sd tile; r is applied once per 32-target group.
"""

import sys

if "/opt/trn_rl_repo" not in sys.path:
    sys.path.insert(0, "/opt/trn_rl_repo")

import numpy as np
import ml_dtypes

BF16 = ml_dtypes.bfloat16
FP8 = ml_dtypes.float8_e4m3

B = 4096
NCORES = 8
BL = B // NCORES        # 512 rows per core
HB = BL // 2            # 256-column trunk half
DIN = 512
D = 256
T = 424
L = 6
EPS = 1e-5
PAIRS = T // 2          # 212
NG = (T + 31) // 32     # 14 head groups of <=32 targets
NB8 = T // 8            # 53 W1 dma blocks of 8 targets
NPB = 27                # W2 dma blocks of 8 pairs (216 padded)

_cache = {}
_SENT = object()


def _prep(inputs):
    f32 = lambda a: np.ascontiguousarray(np.asarray(a, dtype=np.float32))
    b16 = lambda a: np.ascontiguousarray(np.asarray(a, dtype=np.float32).astype(BF16))
    f8 = lambda a: np.ascontiguousarray(np.asarray(a, dtype=np.float32).astype(FP8))

    x = f32(inputs["x"])
    assert np.all(np.asarray(inputs["ln_g"]) == 1.0), "ln_g != 1 unsupported"
    assert np.all(np.asarray(inputs["ln_b"]) == 0.0), "ln_b != 0 unsupported"
    assert np.all(np.asarray(inputs["tp_ln_g"]) == 1.0), "tp_ln_g != 1 unsupported"
    assert np.all(np.asarray(inputs["tp_ln_b"]) == 0.0), "tp_ln_b != 0 unsupported"

    # ---- projection ----
    Wp = f32(inputs["proj_W"]).reshape(D, DIN)        # [256, 512]
    bp = f32(inputs["proj_b"]).reshape(D)
    WpT = b16(Wp.T)                                   # [512, 256] raw
    wpm_neg = -Wp.mean(0)                             # [512] negated col means
    # broadcast to 128 stationary columns: the c matmul then lands the
    # (negated) feature-mean on every psum partition (128x128 PE shape)
    wpmC = b16(np.repeat(wpm_neg.reshape(4, 128).T[:, :, None], 128, axis=2))
    bpm = float(bp.mean())

    # ---- trunk: collapse attention (seq_len=1) into one affine map ----
    aiW = f32(inputs["attn_in_W"])
    aib = f32(inputs["attn_in_b"])
    aoW = f32(inputs["attn_out_W"])
    aob = f32(inputs["attn_out_b"])
    f1W = f32(inputs["ff_W1"])
    f1b = f32(inputs["ff_b1"])
    f2W = f32(inputs["ff_W2"])
    f2b = f32(inputs["ff_b2"])

    WaT = np.empty((L, D, D), np.float32)
    Wf1T = np.empty((L, D, 4 * D), np.float32)
    Wf2T = np.empty((L, 4 * D, D), np.float32)
    ba = np.empty((L, D), np.float32)
    for i in range(L):
        Wv, bv = aiW[i, 2 * D:], aib[i, 2 * D:]
        Wa = aoW[i] @ Wv
        WaT[i] = (Wa - Wa.mean(0, keepdims=True)).T
        bai = aoW[i] @ bv + aob[i]
        ba[i] = bai - bai.mean()
        Wf1T[i] = f1W[i].T
        Wf2T[i] = (f2W[i] - f2W[i].mean(0, keepdims=True)).T
    ba0p = ba[0] - bpm                                 # layer-0 merged bias
    bf1 = f1b
    bf2 = f2b - f2b.mean(1, keepdims=True)

    # ---- fold cross-attention (affine) into head W1 ----
    cW = f32(inputs["cross_in_W"])
    P = f32(inputs["cross_out_W"]) @ cW[2 * D:]        # Wco @ Wcv
    bc = f32(inputs["cross_out_W"]) @ f32(inputs["cross_in_b"])[2 * D:] \
        + f32(inputs["cross_out_b"])

    # ---- trunk bias pack: [nb, 128] fp32 -> sbuf [128, nb] ----
    cols = []

    def pack(vec):
        v = f32(vec).reshape(-1, 128)
        s = len(cols)
        cols.extend(v)
        return s

    bias_idx = {
        "bp": pack(bp),
        "ba": [pack(ba0p)] + [pack(ba[i]) for i in range(1, L)],
        "bf1": [pack(bf1[i]) for i in range(L)],
        "bf2": [pack(bf2[i]) for i in range(L)],
    }
    TB = f32(np.stack(cols))                           # [nb, 128]

    # ---- heads (cross folded in) ----
    W1 = f32(inputs["tp_W1"])                          # [424, 128, 256]
    b1 = f32(inputs["tp_b1"])                          # [424, 128]
    W1f = W1.reshape(T * 128, D)
    W1eff = (W1f + W1f @ P).reshape(T, 128, D)
    b1eff = b1 + (W1f @ bc).reshape(T, 128)
    W1c = W1eff - W1eff.mean(1, keepdims=True)
    b1c = b1eff - b1eff.mean(1, keepdims=True)
    b1T = f32(b1c.T)                                   # [128, 424]
    # W1G[gi, k, 2*ti+c, m] = W1c[8gi+ti].T[128c+k, m]
    W1G = b16(np.transpose(
        W1c.transpose(0, 2, 1).reshape(NB8, 8, 2, 128, 128),
        (0, 3, 1, 2, 4)).reshape(NB8, 128, 16, 128))

    # All stationary operands below are zero-padded to a full 128x128 PE
    # tile: switching the PE tile config between matmuls of different
    # (row,col) shapes costs ~110ns on HW, and zero rows/cols are free
    # (cost is output free size only; accumulating zeros is exact).
    W2 = f32(inputs["tp_W2"])                          # [424, 64, 128]
    b2 = f32(inputs["tp_b2"])                          # [424, 64]
    W2P = W2.transpose(0, 2, 1).reshape(PAIRS, 2, 128, 64)  # [212,2,128,64]
    W2Pp = np.zeros((NPB * 8, 2, 128, 128), np.float32)
    W2Pp[:PAIRS, 0, :, 0:64] = W2P[:, 0]               # t0 -> z rows 0:64
    W2Pp[:PAIRS, 1, :, 64:128] = W2P[:, 1]             # t1 -> z rows 64:128
    # W2G[gi, k, qi, e, m]
    W2G = b16(np.transpose(
        W2Pp.reshape(NPB, 8, 2, 128, 128), (0, 3, 1, 2, 4)
    ).reshape(NPB, 128, 2048))

    b2G = np.zeros((NG, 128, 16, 128), np.float32)     # K padded 32->128
    W3 = f32(inputs["tp_W3"])                          # [424, 64]
    b3 = f32(inputs["tp_b3"])                          # [424]
    W3G = np.zeros((NG, 128, 16, 128), np.float32)     # cols padded 32->128
    # diag(b3) per group: matmul against the broadcast sd tile injects
    # b3*sd into the o3 psum, so (o3 + b3*sd)*rstd = o3*rstd + b3
    b3D = np.zeros((NG, 128, 128), np.float32)
    for t in range(T):
        g, lt = t // 32, t % 32
        q, e = lt // 2, lt % 2
        b2G[g, lt, q, 64 * e:64 * e + 64] = b2[t]
        W3G[g, 64 * e:64 * e + 64, q, lt] = W3[t]
        b3D[g, lt, lt] = b3[t]
    b2G = b16(b2G.reshape(NG, 128, 2048))
    W3G = b16(W3G.reshape(NG, 128, 2048))
    b3D = b16(b3D)

    # fp8 DoubleRow stats selector: [k, p, e, m] one-hot m = 2p+e,
    # col-padded to 128 so out spans the full psum bank partitions
    selDR = np.zeros((128, 16, 2, 128), np.float32)
    for p in range(16):
        selDR[:, p, 0, 2 * p] = 1.0
        selDR[:, p, 1, 2 * p + 1] = 1.0
    selDR = f8(selDR.reshape(128, 4096))

    shared = {
        "WpT": WpT, "wpmC": wpmC,
        "WaT": b16(WaT),
        "Wf1T": b16(Wf1T), "Wf2T": b16(Wf2T), "TB": TB,
        "W1G": W1G, "b1T": b1T, "W2G": W2G, "b2G": b2G,
        "W3G": W3G, "b3D": b3D, "selDR": selDR,
    }
    in_maps = []
    for c in range(NCORES):
        m = dict(shared)
        m["xT"] = b16(x[c * BL:(c + 1) * BL].T)        # [512, 512]
        in_maps.append(m)
    return in_maps, TB.shape[0], bias_idx


def _build(nb, bias_idx):
    import concourse.bass as bass
    import concourse.mybir as mybir
    import concourse.tile as tile
    from concourse import bacc
    from concourse.masks import make_identity

    f32 = mybir.dt.float32
    bf = mybir.dt.bfloat16
    fp8 = mybir.dt.float8e4
    Alu = mybir.AluOpType
    Act = mybir.ActivationFunctionType
    DR = mybir.MatmulPerfMode.DoubleRow
    ts = bass.ts

    nc = bacc.Bacc(None, target_bir_lowering=False)
    dr = lambda name, shape, dt=bf: nc.dram_tensor(name, shape, dt,
                                                   kind="ExternalInput")
    xT = dr("xT", [DIN, BL])
    WpT = dr("WpT", [DIN, D])
    wpmC = dr("wpmC", [128, 4 * 128])
    WaT = dr("WaT", [L, D, D])
    Wf1T = dr("Wf1T", [L, D, 4 * D])
    Wf2T = dr("Wf2T", [L, 4 * D, D])
    TB = dr("TB", [nb, 128], f32)
    W1G = dr("W1G", [NB8, 128, 16, 128])
    b1T = dr("b1T", [128, T], f32)
    W2G = dr("W2G", [NPB, 128, 2048])
    b2G = dr("b2G", [NG, 128, 2048])
    W3G = dr("W3G", [NG, 128, 2048])
    b3D = dr("b3D", [NG, 128, 128])
    selDR = dr("selDR", [128, 4096], fp8)
    out = nc.dram_tensor("out", [BL, T], f32, kind="ExternalOutput")

    from contextlib import ExitStack

    with tile.TileContext(nc) as tc, ExitStack() as stack:
        consts = stack.enter_context(tc.tile_pool(name="consts", bufs=1))

        tb_sb = consts.tile([128, nb], f32, tag="tb")
        nc.gpsimd.dma_start(out=tb_sb, in_=TB.rearrange("n p -> p n"))
        # heads-only constants: allocate now, DMA later (after trunk inputs)
        b1_sb = consts.tile([128, T], f32, tag="b1")
        sel_sb = consts.tile([128, 16, 2, 128], fp8, tag="sel")
        eps_col = consts.tile([128, 1], f32, tag="eps")
        nc.vector.memset(eps_col, EPS)
        # warm the ACT table (sqrt+square+identity+relu) during input DMAs
        warm = consts.tile([1, 1], f32, tag="warm")
        nc.scalar.activation(warm, eps_col[0:1, 0:1], Act.Sqrt, bias=0.0,
                             scale=1.0)
        ones_f = consts.tile([128, 128], bf, tag="onesf")
        nc.vector.memset(ones_f, 1.0)
        idnb = consts.tile([128, 128], bf, tag="idnb")
        make_identity(nc, idnb)
        # bf16 transposed output staging: [batch 128, b-block 4, padded tgts]
        out16 = consts.tile([128, 4, 32 * NG + 16], bf, tag="o16")
        out_sb = [consts.tile([128, T], f32, tag=f"ob{i}", name=f"ob{i}")
                  for i in range(4)]
        hcs = [consts.tile([128, BL], bf, tag=f"hc{m}", name=f"hc{m}")
               for m in range(2)]

        def bias_col(idx, m=0):
            return tb_sb[:, idx + m:idx + m + 1]

        # ================= trunk =================
        with tc.tile_pool(name="twt", bufs=2) as twt, \
             tc.tile_pool(name="tact", bufs=2) as tact, \
             tc.tile_pool(name="hpool", bufs=2) as hpool, \
             tc.tile_pool(name="tring", bufs=3, space="PSUM") as tring, \
             tc.tile_pool(name="tsq", bufs=2, space="PSUM") as tsq:

            xs = twt.tile([128, 4, BL], bf, tag="x", bufs=1)
            nc.gpsimd.dma_start(out=xs, in_=xT.rearrange("(c k) b -> k c b", c=4))
            wp = twt.tile([128, 4, D], bf, tag="wp", bufs=1)
            nc.gpsimd.dma_start(out=wp, in_=WpT.rearrange("(c k) m -> k c m", c=4))
            wpm_sb = twt.tile([128, 4, 128], bf, tag="wpm", bufs=1)
            nc.gpsimd.dma_start(out=wpm_sb,
                                in_=wpmC.rearrange("p (k m) -> p k m", k=4))
            # heads constants stream in behind the trunk-critical inputs
            nc.gpsimd.dma_start(out=b1_sb, in_=b1T[:, :])
            nc.gpsimd.dma_start(out=sel_sb,
                                in_=selDR.rearrange("p (q e m) -> p q e m",
                                                    q=16, e=2))

            wts = {}

            def get_wt(key, shape, src, c):
                if key not in wts:
                    w = twt.tile([128, c, shape], bf, tag=key[0], name=key[0])
                    nc.gpsimd.dma_start(
                        out=w, in_=src.rearrange("(c k) m -> k c m", c=c))
                    wts[key] = w
                return wts[key]

            def half_gen(s):
                cs = slice(HB * s, HB * s + HB)
                # ---- proj matmuls (c lands broadcast on all partitions) ----
                cps = tsq.tile([128, HB], f32, tag="ssq", name=f"cps{s}",
                               padded_shape=[128, 512])
                for k in range(4):
                    nc.tensor.matmul(cps, wpm_sb[:, k, :], xs[:, k, cs],
                                     start=(k == 0), stop=(k == 3))
                hps = tring.tile([128, 2, HB], f32, tag=f"rg{s}", name=f"hps{s}")
                for m in range(2):
                    for k in range(4):
                        nc.tensor.matmul(hps[:, m, :], wp[:, k, ts(m, 128)],
                                         xs[:, k, cs],
                                         start=(m == 0 and k == 0),
                                         stop=(m == 1 and k == 3))
                yield
                # ---- proj finish ----
                cneg = tact.tile([128, HB], bf, tag=f"cn{s}", name=f"cn{s}")
                nc.scalar.copy(cneg, cps)
                h = hpool.tile([128, 2, HB], bf, tag=f"h{s}", name=f"h{s}")
                for m in range(2):
                    nc.scalar.activation(h[:, m, :], hps[:, m, :], Act.Identity,
                                         bias=bias_col(bias_idx["bp"], m),
                                         scale=1.0)
                yield

                def ln_a(yps, bidx):
                    """Materialize yp = psum + bias (bf16), square on DVE
                    (2x mode, keeps ACT free for gelu/sqrt), then matmul
                    against a full-ones stationary so the psum output IS the
                    partition-broadcast sum of squares (128x128 PE shape)."""
                    yp = tact.tile([128, 2, HB], bf, tag=f"yp{s}",
                                   name=f"yp{s}")
                    sq = tact.tile([128, 2, HB], bf, tag=f"sq{s}", name=f"sq{s}")
                    for m in range(2):
                        nc.vector.tensor_scalar(
                            out=yp[:, m, :], in0=yps[:, m, :],
                            scalar1=bias_col(bidx, m), scalar2=None,
                            op0=Alu.add)
                        nc.vector.tensor_tensor(out=sq[:, m, :],
                                                in0=yp[:, m, :],
                                                in1=yp[:, m, :], op=Alu.mult)
                    ssq = tsq.tile([128, HB], f32, tag="ssq", name=f"ssq{s}",
                                   padded_shape=[128, 512])
                    for m in range(2):
                        nc.tensor.matmul(ssq, ones_f, sq[:, m, :],
                                         start=(m == 0), stop=(m == 1))
                    return yp, ssq

                def ln_b(yp, ssq, bidx, houts, warm_next):
                    """houts: list of two [128, HB] APs (m-chunk outputs).
                    ssq is partition-broadcast, so sqrt/recip directly yield
                    the broadcast rstd -- no bcast matmul, no bf16 copy."""
                    sd = tact.tile([128, HB], f32, tag=f"sd{s}", name=f"sd{s}")
                    nc.scalar.activation(sd, ssq, Act.Sqrt,
                                         bias=eps_col, scale=1.0 / D)
                    if warm_next is not None:
                        # preload the next ACT table; reading sd anchors the
                        # load here (no-dep warms get hoisted to t=0 and
                        # thrash the table back and forth)
                        nc.scalar.activation(warm, sd[0:1, 0:1],
                                             warm_next, bias=0.0, scale=0.0)
                    r = tact.tile([128, HB], f32, tag=f"r{s}", name=f"r{s}")
                    nc.vector.reciprocal_approx_fast(out=r, in_=sd)
                    for m in range(2):
                        nc.vector.tensor_tensor(out=houts[m], in0=yp[:, m, :],
                                                in1=r, op=Alu.mult)

                for i in range(L):
                    # ---- attn (collapsed affine) + residual ----
                    wa = get_wt(("wa", i), D, WaT[i], 2)
                    yps = tring.tile([128, 2, HB], f32, tag=f"rg{s}",
                                     name=f"yps{s}")
                    for m in range(2):
                        for k in range(2):
                            nc.tensor.matmul(yps[:, m, :], wa[:, k, ts(m, 128)],
                                             h[:, k, :],
                                             start=(m == 0 and k == 0),
                                             stop=False)
                        nc.tensor.matmul(yps[:, m, :], idnb, h[:, m, :],
                                         start=False,
                                         stop=(i != 0 and m == 1))
                        if i == 0:
                            nc.tensor.matmul(yps[:, m, :], idnb, cneg,
                                             start=False, stop=(m == 1))
                    yield
                    yp, ssq = ln_a(yps, bias_idx["ba"][i])
                    yield
                    hn = hpool.tile([128, 2, HB], bf, tag=f"h{s}",
                                    name=f"h{s}")
                    ln_b(yp, ssq, bias_idx["ba"][i],
                         [hn[:, 0, :], hn[:, 1, :]],
                         Act.Gelu if s == 1 else None)
                    h = hn
                    yield
                    # ---- feed-forward ----
                    w1 = get_wt(("w1", i), 4 * D, Wf1T[i], 2)
                    g = hpool.tile([128, 8, HB], bf, tag=f"g{s}", name=f"g{s}")
                    for gm in range(4):
                        gps = tring.tile([128, 2, HB], f32, tag=f"rg{s}",
                                         name=f"gps{s}")
                        for hm in range(2):
                            m = 2 * gm + hm
                            for k in range(2):
                                nc.tensor.matmul(gps[:, hm, :],
                                                 w1[:, k, ts(m, 128)],
                                                 h[:, k, :],
                                                 start=(hm == 0 and k == 0),
                                                 stop=(hm == 1 and k == 1))
                        for hm in range(2):
                            m = 2 * gm + hm
                            nc.scalar.activation(g[:, m, :], gps[:, hm, :],
                                                 Act.Gelu,
                                                 bias=bias_col(
                                                     bias_idx["bf1"][i], m),
                                                 scale=1.0)
                    if s == 1:
                        # preload the sqrt table for the upcoming LN2s;
                        # reading g anchors the load after the gelus
                        nc.scalar.activation(warm, g[0:1, 7, 0:1],
                                             Act.Sqrt, bias=0.0, scale=0.0)
                    yield
                    w2 = get_wt(("w2", i), D, Wf2T[i], 8)
                    yps2 = tring.tile([128, 2, HB], f32, tag=f"rg{s}",
                                      name=f"yps2{s}")
                    for m in range(2):
                        for k in range(8):
                            nc.tensor.matmul(yps2[:, m, :], w2[:, k, ts(m, 128)],
                                             g[:, k, :],
                                             start=(m == 0 and k == 0),
                                             stop=False)
                        nc.tensor.matmul(yps2[:, m, :], idnb, h[:, m, :],
                                         start=False, stop=(m == 1))
                    yield
                    yp, ssq = ln_a(yps2, bias_idx["bf2"][i])
                    yield
                    if i == L - 1:
                        ln_b(yp, ssq, bias_idx["bf2"][i],
                             [hcs[0][:, cs], hcs[1][:, cs]], None)
                    else:
                        hn = hpool.tile([128, 2, HB], bf, tag=f"h{s}",
                                        name=f"h{s}")
                        ln_b(yp, ssq, bias_idx["bf2"][i],
                             [hn[:, 0, :], hn[:, 1, :]], None)
                        h = hn
                    yield

            g0 = half_gen(0)
            g1 = half_gen(1)
            NST = 2 + L * 7
            OFF = 0
            for step in range(NST + OFF):
                if step < NST:
                    next(g0)
                if step >= OFF:
                    next(g1)

        # ================= heads =================
        with tc.tile_pool(name="w1p", bufs=4) as w1p, \
             tc.tile_pool(name="w2p", bufs=2) as w2p, \
             tc.tile_pool(name="w3p", bufs=2) as w3p, \
             tc.tile_pool(name="b2p", bufs=2) as b2p, \
             tc.tile_pool(name="sqp", bufs=6) as sqp, \
             tc.tile_pool(name="Rp", bufs=72) as Rp, \
             tc.tile_pool(name="R2p", bufs=6) as R2p, \
             tc.tile_pool(name="grp", bufs=2) as grp, \
             tc.tile_pool(name="Tps", bufs=2, space="PSUM") as Tps, \
             tc.tile_pool(name="Sps", bufs=1, space="PSUM") as Sps, \
             tc.tile_pool(name="Zps", bufs=2, space="PSUM") as Zps, \
             tc.tile_pool(name="Ops", bufs=1, space="PSUM") as Ops:

            w1t_ref = [None]
            w2t_ref = [None]
            states = {}

            def phase_A(g):
                """W1 pair psums, fp8 squares, relu, DoubleRow stats; stats
                lag two pairs behind so the PE queue is not blocked on ACT.
                Yields after each pair so phase_B of the previous group can
                interleave at pair granularity."""
                gs = min(32, T - 32 * g)
                npair = gs // 2
                ssq = Sps.tile([128, BL], f32, tag="ssq", name="ssq")
                Rlist = []
                pend = []                        # (p, sqpair) awaiting stats
                for p in range(npair):
                    t0 = 32 * g + 2 * p
                    if t0 % 8 == 0:
                        w1t_ref[0] = w1p.tile([128, 16, 128], bf,
                                              tag="w1", name="w1")
                        nc.gpsimd.dma_start(out=w1t_ref[0], in_=W1G[t0 // 8])
                    w1t = w1t_ref[0]
                    Tp = Tps.tile([128, 2, BL], f32, tag="T", name="Tps")
                    # chunk-major across the two targets: consecutive matmuls
                    # hit different psum banks so fills overlap drains
                    for k in range(2):
                        for e in range(2):
                            t = t0 + e
                            nc.tensor.matmul(
                                Tp[:, e, :], w1t[:, 2 * (t % 8) + k, :], hcs[k],
                                start=(k == 0), stop=(k == 1))
                    sqpair = sqp.tile([128, 2, BL], fp8, tag="sq", name="sq")
                    for e in range(2):
                        t = t0 + e
                        nc.scalar.activation(sqpair[:, e, :], Tp[:, e, :],
                                             Act.Square,
                                             bias=b1_sb[:, t:t + 1], scale=1.0)
                        R = Rp.tile([128, BL], bf, tag="R", name="R")
                        nc.vector.tensor_scalar(
                            out=R, in0=Tp[:, e, :], scalar1=b1_sb[:, t:t + 1],
                            scalar2=0.0, op0=Alu.add, op1=Alu.max)
                        Rlist.append(R)
                    pend.append((p, sqpair))
                    while len(pend) > 3:         # stats lag three pairs
                        j, sqj = pend.pop(0)
                        nc.tensor.matmul(ssq, sel_sb[:, j, :, :], sqj,
                                         start=(j == 0), stop=False,
                                         perf_mode=DR)
                    yield
                for n, (j, sqj) in enumerate(pend):
                    nc.tensor.matmul(ssq, sel_sb[:, j, :, :], sqj,
                                     start=(j == 0), stop=(n == len(pend) - 1),
                                     perf_mode=DR)
                sdf = grp.tile([32, BL], f32, tag="sdf", name="sdf")
                nc.scalar.activation(sdf, ssq[0:32, :], Act.Sqrt,
                                     bias=eps_col[0:32], scale=1.0 / 128)
                # bf16 sd on all 128 partitions straight from the psum (the
                # zero-padded rows give sqrt(eps); b2G's zero rows null them)
                sdb = grp.tile([128, BL], bf, tag="sdb", name="sdb")
                nc.scalar.activation(sdb, ssq, Act.Sqrt,
                                     bias=eps_col, scale=1.0 / 128)
                rstd = grp.tile([32, BL], f32, tag="rst", name="rstd")
                nc.vector.reciprocal_approx_fast(out=rstd, in_=sdf)
                # prefetch phase-B weights for this group
                w3t = w3p.tile([128, 16, 128], bf, tag="w3", name="w3")
                nc.gpsimd.dma_start(out=w3t, in_=W3G[g].rearrange(
                    "p (q m) -> p q m", q=16))
                b2t = b2p.tile([128, 16, 128], bf, tag="b2", name="b2")
                nc.gpsimd.dma_start(out=b2t, in_=b2G[g].rearrange(
                    "p (q m) -> p q m", q=16))
                b3dt = b2p.tile([128, 128], bf, tag="b3d", name="b3d")
                nc.gpsimd.dma_start(out=b3dt, in_=b3D[g])
                states[g] = (Rlist, sdb, rstd, w3t, b2t, b3dt)

            def phase_B(g):
                """Z (+ b2 (x) sd), relu, W3 accumulation, output block.
                Yields after each 2-pair step."""
                Rlist, sdb, rstd, w3t, b2t, b3dt = states.pop(g)
                gs = min(32, T - 32 * g)
                npair = gs // 2
                o3g = Ops.tile([128, BL], f32, tag="o3g", name="o3g")
                nc.tensor.matmul(o3g, b3dt, sdb, start=True, stop=False)
                pend = []                       # (q, R2) awaiting W3 mm
                for qb in range(0, npair, 2):
                    qs = [qb] + ([qb + 1] if qb + 1 < npair else [])
                    if (16 * g + qb) % 8 == 0:
                        w2t_ref[0] = w2p.tile([128, 8, 2, 128], bf,
                                              tag="w2", name="w2")
                        nc.gpsimd.dma_start(
                            out=w2t_ref[0], in_=W2G[(16 * g + qb) // 8].rearrange(
                                "p (q e m) -> p q e m", q=8, e=2))
                    w2t = w2t_ref[0]
                    # two pairs interleaved: consecutive matmuls alternate
                    # between the two zps banks so fills overlap drains
                    zl = [Zps.tile([128, BL], f32, tag="z", name="zps")
                          for _ in qs]
                    for q, z in zip(qs, zl):
                        nc.tensor.matmul(z, b2t[:, q, :], sdb,
                                         start=True, stop=False)
                    for q, z in zip(qs, zl):
                        nc.tensor.matmul(z, w2t[:, (16 * g + q) % 8, 0, :],
                                         Rlist[2 * q], start=False, stop=False)
                    for q, z in zip(qs, zl):
                        nc.tensor.matmul(z, w2t[:, (16 * g + q) % 8, 1, :],
                                         Rlist[2 * q + 1], start=False, stop=True)
                    for q, z in zip(qs, zl):
                        R2 = R2p.tile([128, BL], bf, tag="R2", name="R2")
                        if q % 2 == 0:
                            nc.scalar.activation(R2, z, Act.Relu, bias=0.0,
                                                 scale=1.0)
                        else:
                            nc.vector.tensor_scalar(out=R2, in0=z, scalar1=0.0,
                                                    scalar2=None, op0=Alu.max)
                        pend.append((q, R2))
                    while len(pend) > 2:        # W3 lag ~2 pairs
                        j, R2j = pend.pop(0)
                        nc.tensor.matmul(o3g, w3t[:, j, :], R2j,
                                         start=False, stop=False)
                    yield
                for n, (j, R2j) in enumerate(pend):
                    nc.tensor.matmul(o3g, w3t[:, j, :], R2j,
                                     start=False, stop=(n == len(pend) - 1))

                # final: scale by rstd (b3 rode the psum as b3*sd), then
                # xbar-transpose to [batch, tgt]
                o3u = grp.tile([32, BL], bf, tag="o3u", name="o3u")
                nc.vector.tensor_tensor(out=o3u, in0=o3g[0:32, :], in1=rstd,
                                        op=Alu.mult)
                rows = gs if gs % 16 == 0 else 16
                nc.sync.dma_start_transpose(
                    out16[:, :, 32 * g:32 * g + rows], o3u[0:rows, :])

            def drain(gen):
                for _ in gen:
                    pass

            drain(phase_A(0))
            for g in range(1, NG):
                gA, gB = phase_A(g), phase_B(g - 1)
                done_a = done_b = False
                while not (done_a and done_b):
                    for _ in range(2):
                        if not done_a:
                            done_a = next(gA, _SENT) is _SENT
                    if not done_b:
                        done_b = next(gB, _SENT) is _SENT
            drain(phase_B(NG - 1))

            for bc in range(4):
                nc.vector.tensor_copy(out=out_sb[bc], in_=out16[:, bc, 0:T])
                nc.gpsimd.dma_start(out=out[ts(bc, 128)], in_=out_sb[bc])

    nc.compile()
    return nc


def kernel(**inputs):
    from concourse.bass_utils import run_bass_kernel_spmd

    in_maps, nb, bias_idx = _prep(inputs)
    if "nc" not in _cache:
        _cache["nc"] = _build(nb, bias_idx)
    nc = _cache["nc"]
    import os
    res = run_bass_kernel_spmd(
        nc, in_maps, core_ids=list(range(NCORES)),
        trace=bool(int(os.environ.get("KTRACE", "0"))))
    _cache["last_result"] = res
    outs = [np.asarray(r["out"], dtype=np.float32) for r in res.results]
    return np.concatenate(outs, axis=0)


# revision 57
# speedup vs baseline: 1.0786x; 1.0786x over previous
"""AdvancedFeatureTransformer Trainium2 kernel (bf16 + fp8 stats, interleaved).

Data-parallel over batch: 8 cores x 512 rows, no collectives.
All activations feature-major (h^T: [feat_part, batch_free]); matmul
operands bf16 (1 cycle/row), psum accumulates fp32.

Key structure (v3):
  - Attention at seq_len=1 is affine: Wa = Wo@Wv, ba = Wo@bv + bo are
    collapsed on the host, removing the v-projection stage per layer.
  - The cross-attention block (also affine) is folded into the 424 head
    W1 matrices on the host: W1eff = W1@(I + Wco@Wcv).
  - EVERY matmul is a full (128,128) PE tile: small stationaries (trunk
    LN stats/bcast, head W2/W3/b2/selDR) are zero-padded, because the HW
    charges ~110ns to switch PE tile configs between matmuls of
    different (row,col) shapes (measured 216ns same-shape vs 330ns
    switching for 512-col bf16 matmuls).
  - Trunk runs two 256-column batch halves interleaved one layer apart,
    so the halves' sqrt/gelu table uses cluster (2 ACT table loads per
    layer, hoisted off the LN chain by data-anchored warm activations).
  - Trunk LN stats matmul vs a full-ones stationary makes the psum
    output the partition-broadcast sum of squares, so sqrt/recip give
    broadcast rstd directly (no bcast matmul, no bf16 round trip).
  - Layer-0 LN mean-correction (-c) rides the attn psum via an idnb
    matmul of the broadcast c tile (no gpsimd broadcast).
  - Head LN stats use fp8e4 squares and a DoubleRow selection matmul:
    one instruction streams TWO targets' squares at 2 cols/cycle.
  - Head W1 psum is a [128, 2, 512] pair tile (fewer semaphore waits);
    relu on DVE, square (fp8 out) on ACT read the psum directly; phase
    A of group g is interleaved with phase B of group g-1 at pair
    granularity to balance PE/ACT/DVE.
  - rstd deferral for heads: relu(Z*r + b2) = r*relu(Z + b2*sd); the
    rank-1 b2 (x) sd term is a (zero-padded) K=128 matmul against the
    bank-broadcast sd tile; r is applied once per 32-target group.
"""

import sys

if "/opt/trn_rl_repo" not in sys.path:
    sys.path.insert(0, "/opt/trn_rl_repo")

import numpy as np
import ml_dtypes

BF16 = ml_dtypes.bfloat16
FP8 = ml_dtypes.float8_e4m3

B = 4096
NCORES = 8
BL = B // NCORES        # 512 rows per core
HB = BL // 2            # 256-column trunk half
DIN = 512
D = 256
T = 424
L = 6
EPS = 1e-5
PAIRS = T // 2          # 212
NG = (T + 31) // 32     # 14 head groups of <=32 targets
NB8 = T // 8            # 53 W1 dma blocks of 8 targets
NPB = 27                # W2 dma blocks of 8 pairs (216 padded)

_cache = {}
_SENT = object()


def _prep(inputs):
    f32 = lambda a: np.ascontiguousarray(np.asarray(a, dtype=np.float32))
    b16 = lambda a: np.ascontiguousarray(np.asarray(a, dtype=np.float32).astype(BF16))
    f8 = lambda a: np.ascontiguousarray(np.asarray(a, dtype=np.float32).astype(FP8))

    x = f32(inputs["x"])
    assert np.all(np.asarray(inputs["ln_g"]) == 1.0), "ln_g != 1 unsupported"
    assert np.all(np.asarray(inputs["ln_b"]) == 0.0), "ln_b != 0 unsupported"
    assert np.all(np.asarray(inputs["tp_ln_g"]) == 1.0), "tp_ln_g != 1 unsupported"
    assert np.all(np.asarray(inputs["tp_ln_b"]) == 0.0), "tp_ln_b != 0 unsupported"

    # ---- projection ----
    Wp = f32(inputs["proj_W"]).reshape(D, DIN)        # [256, 512]
    bp = f32(inputs["proj_b"]).reshape(D)
    WpT = b16(Wp.T)                                   # [512, 256] raw
    wpm_neg = -Wp.mean(0)                             # [512] negated col means
    # broadcast to 128 stationary columns: the c matmul then lands the
    # (negated) feature-mean on every psum partition (128x128 PE shape)
    wpmC = b16(np.repeat(wpm_neg.reshape(4, 128).T[:, :, None], 128, axis=2))
    bpm = float(bp.mean())

    # ---- trunk: collapse attention (seq_len=1) into one affine map ----
    aiW = f32(inputs["attn_in_W"])
    aib = f32(inputs["attn_in_b"])
    aoW = f32(inputs["attn_out_W"])
    aob = f32(inputs["attn_out_b"])
    f1W = f32(inputs["ff_W1"])
    f1b = f32(inputs["ff_b1"])
    f2W = f32(inputs["ff_W2"])
    f2b = f32(inputs["ff_b2"])

    WaT = np.empty((L, D, D), np.float32)
    Wf1T = np.empty((L, D, 4 * D), np.float32)
    Wf2T = np.empty((L, 4 * D, D), np.float32)
    ba = np.empty((L, D), np.float32)
    for i in range(L):
        Wv, bv = aiW[i, 2 * D:], aib[i, 2 * D:]
        Wa = aoW[i] @ Wv
        WaT[i] = (Wa - Wa.mean(0, keepdims=True)).T
        bai = aoW[i] @ bv + aob[i]
        ba[i] = bai - bai.mean()
        Wf1T[i] = f1W[i].T
        Wf2T[i] = (f2W[i] - f2W[i].mean(0, keepdims=True)).T
    ba0p = ba[0] - bpm                                 # layer-0 merged bias
    bf1 = f1b
    bf2 = f2b - f2b.mean(1, keepdims=True)

    # ---- fold cross-attention (affine) into head W1 ----
    cW = f32(inputs["cross_in_W"])
    P = f32(inputs["cross_out_W"]) @ cW[2 * D:]        # Wco @ Wcv
    bc = f32(inputs["cross_out_W"]) @ f32(inputs["cross_in_b"])[2 * D:] \
        + f32(inputs["cross_out_b"])

    # ---- trunk bias pack: [nb, 128] fp32 -> sbuf [128, nb] ----
    cols = []

    def pack(vec):
        v = f32(vec).reshape(-1, 128)
        s = len(cols)
        cols.extend(v)
        return s

    bias_idx = {
        "bp": pack(bp),
        "ba": [pack(ba0p)] + [pack(ba[i]) for i in range(1, L)],
        "bf1": [pack(bf1[i]) for i in range(L)],
        "bf2": [pack(bf2[i]) for i in range(L)],
    }
    TB = f32(np.stack(cols))                           # [nb, 128]

    # ---- heads (cross folded in) ----
    W1 = f32(inputs["tp_W1"])                          # [424, 128, 256]
    b1 = f32(inputs["tp_b1"])                          # [424, 128]
    W1f = W1.reshape(T * 128, D)
    W1eff = (W1f + W1f @ P).reshape(T, 128, D)
    b1eff = b1 + (W1f @ bc).reshape(T, 128)
    W1c = W1eff - W1eff.mean(1, keepdims=True)
    b1c = b1eff - b1eff.mean(1, keepdims=True)
    b1T = f32(b1c.T)                                   # [128, 424]
    # W1G[gi, k, 2*ti+c, m] = W1c[8gi+ti].T[128c+k, m]
    W1G = b16(np.transpose(
        W1c.transpose(0, 2, 1).reshape(NB8, 8, 2, 128, 128),
        (0, 3, 1, 2, 4)).reshape(NB8, 128, 16, 128))

    # All stationary operands below are zero-padded to a full 128x128 PE
    # tile: switching the PE tile config between matmuls of different
    # (row,col) shapes costs ~110ns on HW, and zero rows/cols are free
    # (cost is output free size only; accumulating zeros is exact).
    W2 = f32(inputs["tp_W2"])                          # [424, 64, 128]
    b2 = f32(inputs["tp_b2"])                          # [424, 64]
    W2P = W2.transpose(0, 2, 1).reshape(PAIRS, 2, 128, 64)  # [212,2,128,64]
    W2Pp = np.zeros((NPB * 8, 2, 128, 128), np.float32)
    W2Pp[:PAIRS, 0, :, 0:64] = W2P[:, 0]               # t0 -> z rows 0:64
    W2Pp[:PAIRS, 1, :, 64:128] = W2P[:, 1]             # t1 -> z rows 64:128
    # W2G[gi, k, qi, e, m]
    W2G = b16(np.transpose(
        W2Pp.reshape(NPB, 8, 2, 128, 128), (0, 3, 1, 2, 4)
    ).reshape(NPB, 128, 2048))

    b2G = np.zeros((NG, 128, 16, 128), np.float32)     # K padded 32->128
    W3 = f32(inputs["tp_W3"])                          # [424, 64]
    b3 = f32(inputs["tp_b3"])                          # [424]
    W3G = np.zeros((NG, 128, 16, 128), np.float32)     # cols padded 32->128
    b3B = np.zeros((32, NG), np.float32)
    for t in range(T):
        g, lt = t // 32, t % 32
        q, e = lt // 2, lt % 2
        b2G[g, lt, q, 64 * e:64 * e + 64] = b2[t]
        W3G[g, 64 * e:64 * e + 64, q, lt] = W3[t]
        b3B[lt, g] = b3[t]
    b2G = b16(b2G.reshape(NG, 128, 2048))
    W3G = b16(W3G.reshape(NG, 128, 2048))
    b3B = f32(b3B)

    # fp8 DoubleRow stats selector: [k, p, e, m] one-hot m = 2p+e,
    # col-padded to 128 so out spans the full psum bank partitions
    selDR = np.zeros((128, 16, 2, 128), np.float32)
    for p in range(16):
        selDR[:, p, 0, 2 * p] = 1.0
        selDR[:, p, 1, 2 * p + 1] = 1.0
    selDR = f8(selDR.reshape(128, 4096))

    # head-LN stats via the square expansion: sum (T+b1)^2 =
    # sum T^2 + (2 W1c^T b1c) . h + sum b1^2 -- so the fp8 squares need
    # no per-target bias (one ACT op per PAIR), the cross term is a
    # 2-matmul-per-group correction into the ssq psum, and the constant
    # rides the sqrt bias.
    wtl = 2.0 * np.einsum('tkd,tk->td', W1c, b1c)      # [424, 256]
    WtG = np.zeros((NG, 128, 2, 128), np.float32)
    constT = (b1c ** 2).sum(1)                         # [424]
    biasG = np.full((128, NG), EPS, np.float32)
    for t in range(T):
        g, lt = t // 32, t % 32
        for kc in range(2):
            WtG[g, :, kc, lt] = wtl[t, 128 * kc:128 * kc + 128]
        biasG[lt, g] = constT[t] / 128.0 + EPS
    WtG = b16(WtG.reshape(NG, 128, 256))
    biasG = f32(biasG)

    shared = {
        "WpT": WpT, "wpmC": wpmC,
        "WaT": b16(WaT),
        "Wf1T": b16(Wf1T), "Wf2T": b16(Wf2T), "TB": TB,
        "W1G": W1G, "b1T": b1T, "W2G": W2G, "b2G": b2G,
        "W3G": W3G, "b3B": b3B, "selDR": selDR, "WtG": WtG,
        "biasG": biasG,
    }
    in_maps = []
    for c in range(NCORES):
        m = dict(shared)
        m["xT"] = b16(x[c * BL:(c + 1) * BL].T)        # [512, 512]
        in_maps.append(m)
    return in_maps, TB.shape[0], bias_idx


def _build(nb, bias_idx):
    import concourse.bass as bass
    import concourse.mybir as mybir
    import concourse.tile as tile
    from concourse import bacc
    from concourse.masks import make_identity

    f32 = mybir.dt.float32
    bf = mybir.dt.bfloat16
    fp8 = mybir.dt.float8e4
    Alu = mybir.AluOpType
    Act = mybir.ActivationFunctionType
    DR = mybir.MatmulPerfMode.DoubleRow
    ts = bass.ts

    nc = bacc.Bacc(None, target_bir_lowering=False)
    dr = lambda name, shape, dt=bf: nc.dram_tensor(name, shape, dt,
                                                   kind="ExternalInput")
    xT = dr("xT", [DIN, BL])
    WpT = dr("WpT", [DIN, D])
    wpmC = dr("wpmC", [128, 4 * 128])
    WaT = dr("WaT", [L, D, D])
    Wf1T = dr("Wf1T", [L, D, 4 * D])
    Wf2T = dr("Wf2T", [L, 4 * D, D])
    TB = dr("TB", [nb, 128], f32)
    W1G = dr("W1G", [NB8, 128, 16, 128])
    b1T = dr("b1T", [128, T], f32)
    W2G = dr("W2G", [NPB, 128, 2048])
    b2G = dr("b2G", [NG, 128, 2048])
    W3G = dr("W3G", [NG, 128, 2048])
    b3B = dr("b3B", [32, NG], f32)
    selDR = dr("selDR", [128, 4096], fp8)
    WtG = dr("WtG", [NG, 128, 256])
    biasG = dr("biasG", [128, NG], f32)
    out = nc.dram_tensor("out", [BL, T], f32, kind="ExternalOutput")

    from contextlib import ExitStack

    with tile.TileContext(nc) as tc, ExitStack() as stack:
        consts = stack.enter_context(tc.tile_pool(name="consts", bufs=1))

        tb_sb = consts.tile([128, nb], f32, tag="tb")
        nc.gpsimd.dma_start(out=tb_sb, in_=TB.rearrange("n p -> p n"))
        # heads-only constants: allocate now, DMA later (after trunk inputs)
        b1_sb = consts.tile([128, T], f32, tag="b1")
        b3_sb = consts.tile([32, NG], f32, tag="b3")
        bg_sb = consts.tile([128, NG], f32, tag="bg")
        sel_sb = consts.tile([128, 16, 2, 128], fp8, tag="sel")
        eps_col = consts.tile([128, 1], f32, tag="eps")
        nc.vector.memset(eps_col, EPS)
        # warm the ACT table (sqrt+square+identity+relu) during input DMAs
        warm = consts.tile([1, 1], f32, tag="warm")
        nc.scalar.activation(warm, eps_col[0:1, 0:1], Act.Sqrt, bias=0.0,
                             scale=1.0)
        ones_f = consts.tile([128, 128], bf, tag="onesf")
        nc.vector.memset(ones_f, 1.0)
        idnb = consts.tile([128, 128], bf, tag="idnb")
        make_identity(nc, idnb)
        # bf16 transposed output staging: [batch 128, b-block 4, padded tgts]
        out16 = consts.tile([128, 4, 32 * NG + 16], bf, tag="o16")
        out_sb = [consts.tile([128, T], f32, tag=f"ob{i}", name=f"ob{i}")
                  for i in range(4)]
        hcs = [consts.tile([128, BL], bf, tag=f"hc{m}", name=f"hc{m}")
               for m in range(2)]

        def bias_col(idx, m=0):
            return tb_sb[:, idx + m:idx + m + 1]

        # ================= trunk =================
        with tc.tile_pool(name="twt", bufs=2) as twt, \
             tc.tile_pool(name="tact", bufs=2) as tact, \
             tc.tile_pool(name="hpool", bufs=2) as hpool, \
             tc.tile_pool(name="tring", bufs=3, space="PSUM") as tring, \
             tc.tile_pool(name="tsq", bufs=2, space="PSUM") as tsq:

            xs = twt.tile([128, 4, BL], bf, tag="x", bufs=1)
            nc.gpsimd.dma_start(out=xs, in_=xT.rearrange("(c k) b -> k c b", c=4))
            wp = twt.tile([128, 4, D], bf, tag="wp", bufs=1)
            nc.gpsimd.dma_start(out=wp, in_=WpT.rearrange("(c k) m -> k c m", c=4))
            wpm_sb = twt.tile([128, 4, 128], bf, tag="wpm", bufs=1)
            nc.gpsimd.dma_start(out=wpm_sb,
                                in_=wpmC.rearrange("p (k m) -> p k m", k=4))
            # heads constants stream in behind the trunk-critical inputs
            nc.gpsimd.dma_start(out=b1_sb, in_=b1T[:, :])
            nc.gpsimd.dma_start(out=b3_sb, in_=b3B[:, :])
            nc.gpsimd.dma_start(out=bg_sb, in_=biasG[:, :])
            nc.gpsimd.dma_start(out=sel_sb,
                                in_=selDR.rearrange("p (q e m) -> p q e m",
                                                    q=16, e=2))

            wts = {}

            def get_wt(key, shape, src, c):
                if key not in wts:
                    w = twt.tile([128, c, shape], bf, tag=key[0], name=key[0])
                    nc.gpsimd.dma_start(
                        out=w, in_=src.rearrange("(c k) m -> k c m", c=c))
                    wts[key] = w
                return wts[key]

            def half_gen(s):
                cs = slice(HB * s, HB * s + HB)
                # ---- proj matmuls (c lands broadcast on all partitions) ----
                cps = tsq.tile([128, HB], f32, tag="ssq", name=f"cps{s}",
                               padded_shape=[128, 512])
                for k in range(4):
                    nc.tensor.matmul(cps, wpm_sb[:, k, :], xs[:, k, cs],
                                     start=(k == 0), stop=(k == 3))
                hps = tring.tile([128, 2, HB], f32, tag=f"rg{s}", name=f"hps{s}")
                for m in range(2):
                    for k in range(4):
                        nc.tensor.matmul(hps[:, m, :], wp[:, k, ts(m, 128)],
                                         xs[:, k, cs],
                                         start=(m == 0 and k == 0),
                                         stop=(m == 1 and k == 3))
                yield
                # ---- proj finish ----
                cneg = tact.tile([128, HB], bf, tag=f"cn{s}", name=f"cn{s}")
                nc.scalar.copy(cneg, cps)
                h = hpool.tile([128, 2, HB], bf, tag=f"h{s}", name=f"h{s}")
                for m in range(2):
                    nc.scalar.activation(h[:, m, :], hps[:, m, :], Act.Identity,
                                         bias=bias_col(bias_idx["bp"], m),
                                         scale=1.0)
                yield

                def ln_a(yps, bidx):
                    """Materialize yp = psum + bias (bf16), square on DVE
                    (2x mode, keeps ACT free for gelu/sqrt), then matmul
                    against a full-ones stationary so the psum output IS the
                    partition-broadcast sum of squares (128x128 PE shape)."""
                    yp = tact.tile([128, 2, HB], bf, tag=f"yp{s}",
                                   name=f"yp{s}")
                    sq = tact.tile([128, 2, HB], bf, tag=f"sq{s}", name=f"sq{s}")
                    for m in range(2):
                        nc.vector.tensor_scalar(
                            out=yp[:, m, :], in0=yps[:, m, :],
                            scalar1=bias_col(bidx, m), scalar2=None,
                            op0=Alu.add)
                        nc.vector.tensor_tensor(out=sq[:, m, :],
                                                in0=yp[:, m, :],
                                                in1=yp[:, m, :], op=Alu.mult)
                    ssq = tsq.tile([128, HB], f32, tag="ssq", name=f"ssq{s}",
                                   padded_shape=[128, 512])
                    for m in range(2):
                        nc.tensor.matmul(ssq, ones_f, sq[:, m, :],
                                         start=(m == 0), stop=(m == 1))
                    return yp, ssq

                def ln_b(yp, ssq, bidx, houts, warm_next):
                    """houts: list of two [128, HB] APs (m-chunk outputs).
                    ssq is partition-broadcast, so sqrt/recip directly yield
                    the broadcast rstd -- no bcast matmul, no bf16 copy."""
                    sd = tact.tile([128, HB], f32, tag=f"sd{s}", name=f"sd{s}")
                    nc.scalar.activation(sd, ssq, Act.Sqrt,
                                         bias=eps_col, scale=1.0 / D)
                    if warm_next is not None:
                        # preload the next ACT table; reading sd anchors the
                        # load here (no-dep warms get hoisted to t=0 and
                        # thrash the table back and forth)
                        nc.scalar.activation(warm, sd[0:1, 0:1],
                                             warm_next, bias=0.0, scale=0.0)
                    r = tact.tile([128, HB], f32, tag=f"r{s}", name=f"r{s}")
                    nc.vector.reciprocal_approx_fast(out=r, in_=sd)
                    for m in range(2):
                        nc.vector.tensor_tensor(out=houts[m], in0=yp[:, m, :],
                                                in1=r, op=Alu.mult)

                for i in range(L):
                    # ---- attn (collapsed affine) + residual ----
                    wa = get_wt(("wa", i), D, WaT[i], 2)
                    yps = tring.tile([128, 2, HB], f32, tag=f"rg{s}",
                                     name=f"yps{s}")
                    for m in range(2):
                        for k in range(2):
                            nc.tensor.matmul(yps[:, m, :], wa[:, k, ts(m, 128)],
                                             h[:, k, :],
                                             start=(m == 0 and k == 0),
                                             stop=False)
                        nc.tensor.matmul(yps[:, m, :], idnb, h[:, m, :],
                                         start=False,
                                         stop=(i != 0 and m == 1))
                        if i == 0:
                            nc.tensor.matmul(yps[:, m, :], idnb, cneg,
                                             start=False, stop=(m == 1))
                    yield
                    yp, ssq = ln_a(yps, bias_idx["ba"][i])
                    yield
                    hn = hpool.tile([128, 2, HB], bf, tag=f"h{s}",
                                    name=f"h{s}")
                    ln_b(yp, ssq, bias_idx["ba"][i],
                         [hn[:, 0, :], hn[:, 1, :]],
                         Act.Gelu if s == 1 else None)
                    h = hn
                    yield
                    # ---- feed-forward ----
                    w1 = get_wt(("w1", i), 4 * D, Wf1T[i], 2)
                    g = hpool.tile([128, 8, HB], bf, tag=f"g{s}", name=f"g{s}")
                    for gm in range(4):
                        gps = tring.tile([128, 2, HB], f32, tag=f"rg{s}",
                                         name=f"gps{s}")
                        for hm in range(2):
                            m = 2 * gm + hm
                            for k in range(2):
                                nc.tensor.matmul(gps[:, hm, :],
                                                 w1[:, k, ts(m, 128)],
                                                 h[:, k, :],
                                                 start=(hm == 0 and k == 0),
                                                 stop=(hm == 1 and k == 1))
                        for hm in range(2):
                            m = 2 * gm + hm
                            nc.scalar.activation(g[:, m, :], gps[:, hm, :],
                                                 Act.Gelu,
                                                 bias=bias_col(
                                                     bias_idx["bf1"][i], m),
                                                 scale=1.0)
                    if s == 1:
                        # preload the sqrt table for the upcoming LN2s;
                        # reading g anchors the load after the gelus
                        nc.scalar.activation(warm, g[0:1, 7, 0:1],
                                             Act.Sqrt, bias=0.0, scale=0.0)
                    yield
                    w2 = get_wt(("w2", i), D, Wf2T[i], 8)
                    yps2 = tring.tile([128, 2, HB], f32, tag=f"rg{s}",
                                      name=f"yps2{s}")
                    for m in range(2):
                        for k in range(8):
                            nc.tensor.matmul(yps2[:, m, :], w2[:, k, ts(m, 128)],
                                             g[:, k, :],
                                             start=(m == 0 and k == 0),
                                             stop=False)
                        nc.tensor.matmul(yps2[:, m, :], idnb, h[:, m, :],
                                         start=False, stop=(m == 1))
                    yield
                    yp, ssq = ln_a(yps2, bias_idx["bf2"][i])
                    yield
                    if i == L - 1:
                        ln_b(yp, ssq, bias_idx["bf2"][i],
                             [hcs[0][:, cs], hcs[1][:, cs]], None)
                    else:
                        hn = hpool.tile([128, 2, HB], bf, tag=f"h{s}",
                                        name=f"h{s}")
                        ln_b(yp, ssq, bias_idx["bf2"][i],
                             [hn[:, 0, :], hn[:, 1, :]], None)
                        h = hn
                    yield

            g0 = half_gen(0)
            g1 = half_gen(1)
            NST = 2 + L * 7
            OFF = 7
            for step in range(NST + OFF):
                if step < NST:
                    next(g0)
                if step >= OFF:
                    next(g1)

        # ================= heads =================
        with tc.tile_pool(name="w1p", bufs=4) as w1p, \
             tc.tile_pool(name="w2p", bufs=2) as w2p, \
             tc.tile_pool(name="w3p", bufs=2) as w3p, \
             tc.tile_pool(name="b2p", bufs=2) as b2p, \
             tc.tile_pool(name="sqp", bufs=6) as sqp, \
             tc.tile_pool(name="Rp", bufs=72) as Rp, \
             tc.tile_pool(name="R2p", bufs=6) as R2p, \
             tc.tile_pool(name="grp", bufs=2) as grp, \
             tc.tile_pool(name="wtp", bufs=2) as wtp, \
             tc.tile_pool(name="Tps", bufs=2, space="PSUM") as Tps, \
             tc.tile_pool(name="Sps", bufs=1, space="PSUM") as Sps, \
             tc.tile_pool(name="Zps", bufs=2, space="PSUM") as Zps, \
             tc.tile_pool(name="Ops", bufs=1, space="PSUM") as Ops:

            w1t_ref = [None]
            w2t_ref = [None]
            states = {}
            wt_tiles = {}

            def fetch_wt(g):
                wtt = wtp.tile([128, 2, 128], bf, tag="wt", name="wt")
                nc.gpsimd.dma_start(out=wtt, in_=WtG[g].rearrange(
                    "p (c m) -> p c m", c=2))
                wt_tiles[g] = wtt

            fetch_wt(0)

            def phase_A(g):
                """W1 pair psums, fp8 squares, relu, DoubleRow stats; stats
                lag two pairs behind so the PE queue is not blocked on ACT.
                Yields after each pair so phase_B of the previous group can
                interleave at pair granularity."""
                gs = min(32, T - 32 * g)
                npair = gs // 2
                ssq = Sps.tile([128, BL], f32, tag="ssq", name="ssq")
                # cross-term correction (2 W1c^T b1c) . h opens the bank
                wtt = wt_tiles.pop(g)
                for kc in range(2):
                    nc.tensor.matmul(ssq, wtt[:, kc, :], hcs[kc],
                                     start=(kc == 0), stop=False)
                Rlist = []
                pend = []                        # (p, sqpair) awaiting stats
                for p in range(npair):
                    t0 = 32 * g + 2 * p
                    if t0 % 8 == 0:
                        w1t_ref[0] = w1p.tile([128, 16, 128], bf,
                                              tag="w1", name="w1")
                        nc.gpsimd.dma_start(out=w1t_ref[0], in_=W1G[t0 // 8])
                    w1t = w1t_ref[0]
                    Tp = Tps.tile([128, 2, BL], f32, tag="T", name="Tps")
                    # chunk-major across the two targets: consecutive matmuls
                    # hit different psum banks so fills overlap drains
                    for k in range(2):
                        for e in range(2):
                            t = t0 + e
                            nc.tensor.matmul(
                                Tp[:, e, :], w1t[:, 2 * (t % 8) + k, :], hcs[k],
                                start=(k == 0), stop=(k == 1))
                    sqpair = sqp.tile([128, 2, BL], fp8, tag="sq", name="sq")
                    # biasless squares: one ACT op covers the pair
                    nc.scalar.activation(sqpair, Tp, Act.Square,
                                         bias=0.0, scale=1.0)
                    for e in range(2):
                        t = t0 + e
                        R = Rp.tile([128, BL], bf, tag="R", name="R")
                        nc.vector.tensor_scalar(
                            out=R, in0=Tp[:, e, :], scalar1=b1_sb[:, t:t + 1],
                            scalar2=0.0, op0=Alu.add, op1=Alu.max)
                        Rlist.append(R)
                    pend.append((p, sqpair))
                    while len(pend) > 3:         # stats lag three pairs
                        j, sqj = pend.pop(0)
                        nc.tensor.matmul(ssq, sel_sb[:, j, :, :], sqj,
                                         start=False, stop=False,
                                         perf_mode=DR)
                    yield
                for n, (j, sqj) in enumerate(pend):
                    nc.tensor.matmul(ssq, sel_sb[:, j, :, :], sqj,
                                     start=False, stop=(n == len(pend) - 1),
                                     perf_mode=DR)
                sdf = grp.tile([32, BL], f32, tag="sdf", name="sdf")
                nc.scalar.activation(sdf, ssq[0:32, :], Act.Sqrt,
                                     bias=bg_sb[0:32, g:g + 1], scale=1.0 / 128)
                # bf16 sd on all 128 partitions straight from the psum (the
                # zero-padded rows give sqrt(eps); b2G's zero rows null them)
                sdb = grp.tile([128, BL], bf, tag="sdb", name="sdb")
                nc.scalar.activation(sdb, ssq, Act.Sqrt,
                                     bias=bg_sb[:, g:g + 1], scale=1.0 / 128)
                rstd = grp.tile([32, BL], f32, tag="rst", name="rstd")
                nc.vector.reciprocal_approx_fast(out=rstd, in_=sdf)
                # prefetch phase-B weights for this group
                w3t = w3p.tile([128, 16, 128], bf, tag="w3", name="w3")
                nc.gpsimd.dma_start(out=w3t, in_=W3G[g].rearrange(
                    "p (q m) -> p q m", q=16))
                b2t = b2p.tile([128, 16, 128], bf, tag="b2", name="b2")
                nc.gpsimd.dma_start(out=b2t, in_=b2G[g].rearrange(
                    "p (q m) -> p q m", q=16))
                if g + 1 < NG:
                    fetch_wt(g + 1)
                states[g] = (Rlist, sdb, rstd, w3t, b2t)

            def phase_B(g):
                """Z (+ b2 (x) sd), relu, W3 accumulation, output block.
                Yields after each 2-pair step."""
                Rlist, sdb, rstd, w3t, b2t = states.pop(g)
                gs = min(32, T - 32 * g)
                npair = gs // 2
                o3g = Ops.tile([128, BL], f32, tag="o3g", name="o3g")
                pend = []                       # (q, R2) awaiting W3 mm
                for qb in range(0, npair, 2):
                    qs = [qb] + ([qb + 1] if qb + 1 < npair else [])
                    if (16 * g + qb) % 8 == 0:
                        w2t_ref[0] = w2p.tile([128, 8, 2, 128], bf,
                                              tag="w2", name="w2")
                        nc.gpsimd.dma_start(
                            out=w2t_ref[0], in_=W2G[(16 * g + qb) // 8].rearrange(
                                "p (q e m) -> p q e m", q=8, e=2))
                    w2t = w2t_ref[0]
                    # two pairs interleaved: consecutive matmuls alternate
                    # between the two zps banks so fills overlap drains
                    zl = [Zps.tile([128, BL], f32, tag="z", name="zps")
                          for _ in qs]
                    for q, z in zip(qs, zl):
                        nc.tensor.matmul(z, b2t[:, q, :], sdb,
                                         start=True, stop=False)
                    for q, z in zip(qs, zl):
                        nc.tensor.matmul(z, w2t[:, (16 * g + q) % 8, 0, :],
                                         Rlist[2 * q], start=False, stop=False)
                    for q, z in zip(qs, zl):
                        nc.tensor.matmul(z, w2t[:, (16 * g + q) % 8, 1, :],
                                         Rlist[2 * q + 1], start=False, stop=True)
                    for q, z in zip(qs, zl):
                        R2 = R2p.tile([128, BL], bf, tag="R2", name="R2")
                        if q % 2 == 0:
                            nc.scalar.activation(R2, z, Act.Relu, bias=0.0,
                                                 scale=1.0)
                        else:
                            nc.vector.tensor_scalar(out=R2, in0=z, scalar1=0.0,
                                                    scalar2=None, op0=Alu.max)
                        pend.append((q, R2))
                    while len(pend) > 2:        # W3 lag ~2 pairs
                        j, R2j = pend.pop(0)
                        nc.tensor.matmul(o3g, w3t[:, j, :], R2j,
                                         start=(j == 0), stop=False)
                    yield
                for n, (j, R2j) in enumerate(pend):
                    nc.tensor.matmul(o3g, w3t[:, j, :], R2j,
                                     start=(j == 0), stop=(n == len(pend) - 1))

                # final: scale by rstd, add b3, xbar-transpose to [batch, tgt]
                o3u = grp.tile([32, BL], bf, tag="o3u", name="o3u")
                nc.vector.tensor_tensor(out=o3u, in0=o3g[0:32, :], in1=rstd,
                                        op=Alu.mult)
                o3f = grp.tile([32, BL], bf, tag="o3f", name="o3f")
                nc.scalar.activation(o3f, o3u, Act.Identity,
                                     bias=b3_sb[0:32, g:g + 1], scale=1.0)
                rows = gs if gs % 16 == 0 else 16
                nc.sync.dma_start_transpose(
                    out16[:, :, 32 * g:32 * g + rows], o3f[0:rows, :])

            def drain(gen):
                for _ in gen:
                    pass

            drain(phase_A(0))
            for g in range(1, NG):
                gA, gB = phase_A(g), phase_B(g - 1)
                done_a = done_b = False
                while not (done_a and done_b):
                    for _ in range(2):
                        if not done_a:
                            done_a = next(gA, _SENT) is _SENT
                    if not done_b:
                        done_b = next(gB, _SENT) is _SENT
            drain(phase_B(NG - 1))

            for bc in range(4):
                nc.vector.tensor_copy(out=out_sb[bc], in_=out16[:, bc, 0:T])
                nc.gpsimd.dma_start(out=out[ts(bc, 128)], in_=out_sb[bc])

    nc.compile()
    return nc


def kernel(**inputs):
    from concourse.bass_utils import run_bass_kernel_spmd

    in_maps, nb, bias_idx = _prep(inputs)
    if "nc" not in _cache:
        _cache["nc"] = _build(nb, bias_idx)
    nc = _cache["nc"]
    import os
    res = run_bass_kernel_spmd(
        nc, in_maps, core_ids=list(range(NCORES)),
        trace=bool(int(os.environ.get("KTRACE", "0"))))
    _cache["last_result"] = res
    outs = [np.asarray(r["out"], dtype=np.float32) for r in res.results]
    return np.concatenate(outs, axis=0)
